# revision 1
# baseline (speedup 1.0000x reference)
"""Trainium2 Bass kernel for GeodesicLMHeadV2 (hyperbolic LM head).

Math:
    norm_v   = ||w_v||
    x[s, v]  = h0[s] * cosh(norm_v) - h_sp[s] . (sinh(norm_v)/norm_v * w_v)
    logits   = -tau * acosh(x)^2

Device strategy (vocab-parallel over 8 cores, V/8 = 4000 per core):
  * GEMM  y = h0*(cosh|w|-1) - h_sp . (sinh|w|/|w|) w   in bf16, fp32 PSUM accum,
    K = 1027 = 3 special rows (hi/lo split of the h0*(cosh-1) rank-1 term,
    which restores fp32-class accuracy) + 1024 spatial rows.
  * x = y + h0 via the free per-partition bias of the ACT engine.
  * acosh(x) = ln(2x) - 1/(4x^2) - O(x^-4)   (x >= ~10 on this data), with
    1/(4x^2) = exp(-2*ln(2x)) so the whole elementwise chain is Ln/Exp/Square:
    one ACT table set (natural_log_exp_and_others), zero table reloads, no sqrt.
  * cosh/sinh of |w| are evaluated on-device as polynomials in |w|^2
    (|w| <= ~0.4), so no sqrt is needed there either.

Host does layout-only staging: shard vocab, transpose to K-major, replicate h.
"""

import os
import numpy as np
from contextlib import ExitStack

B, L, V, N = 2, 2048, 32000, 1024
NCORES = 8
VLOC = V // NCORES          # 4000
S = B * L                   # 4096
KDIM = N + 3                # 1027: rows 0-2 special, rows 3.. spatial
KTILES = 9                  # 8 full 128-row tiles + one 3-row tile
MT = S // 128               # 32 seq tiles
NCHUNK = 512                # matmul free-dim chunk (PSUM bank)
HALF = 2048                 # columns per postprocess half-tile (4 banks)

LAST_EXEC_NS = None
LAST_RESULTS = None
_BUILD_CACHE = {}


def _ktile_rows(k):
    return 128 if k < KTILES - 1 else KDIM - 128 * (KTILES - 1)


def _build(tau):
    import concourse.bacc as bacc
    import concourse.bass as bass
    import concourse.tile as tile
    import concourse.mybir as mybir

    f32 = mybir.dt.float32
    bf16 = mybir.dt.bfloat16
    AF = mybir.ActivationFunctionType
    ALU = mybir.AluOpType

    nc = bacc.Bacc(None, target_bir_lowering=False, debug=False)

    hT = nc.dram_tensor("hT", [KDIM, S], f32, kind="ExternalInput")
    h0c = nc.dram_tensor("h0c", [128, MT], f32, kind="ExternalInput")
    wT = nc.dram_tensor("wT", [KDIM, VLOC], f32, kind="ExternalInput")
    out = nc.dram_tensor("out", [S, VLOC], f32, kind="ExternalOutput")

    neg_tau = -float(tau)

    with ExitStack() as ctx:
        tc = ctx.enter_context(tile.TileContext(nc))

        persist = ctx.enter_context(tc.tile_pool(name="persist", bufs=1))

        # Persistent bf16 operands: hTbf[k] [128, S], WT[k] [128, VLOC]
        hTbf = [persist.tile([128, S], bf16, tag=f"htbf{k}", name=f"htbf{k}")
                for k in range(KTILES)]
        WTb = [persist.tile([128, VLOC], bf16, tag=f"wtb{k}", name=f"wtb{k}")
               for k in range(KTILES)]
        b2 = persist.tile([128, MT], f32, tag="b2")          # 2*h0 per seq tile
        ones_sq = persist.tile([128, 128], f32, tag="ones")  # lhsT for col-sum+bcast

        nc.vector.memset(ones_sq[:], 1.0)

        HW = VLOC // 2   # 2000: W staging half width
        HS = S // 2      # 2048: h staging half width

        # ---------------- preparation ----------------
        with tc.tile_pool(name="prep", bufs=2) as prep, \
             tc.tile_pool(name="prep1", bufs=1) as prep1, \
             tc.tile_pool(name="ppsum", bufs=1, space="PSUM") as ppsum:

            # rneg broadcast to all partitions, built chunkwise below
            rnegb = prep1.tile([128, VLOC], bf16, tag="rnegb")

            # pass 1: per-vocab-column norm^2 = sum_k wT^2, via Square then a
            # matmul with an all-ones [kw,128] lhsT: every output partition
            # receives the same column sum -> n2 lands broadcast in PSUM.
            n2ps = ppsum.tile([128, 4096], f32)
            for k in range(KTILES):
                kw = _ktile_rows(k)
                for h in range(2):
                    wraw = prep.tile([128, HW], f32, tag="raw", name="wraw")
                    nc.sync.dma_start(
                        out=wraw[:kw, :],
                        in_=wT[k * 128:k * 128 + kw, h * HW:(h + 1) * HW])
                    nc.scalar.activation(wraw[:kw, :], wraw[:kw, :], AF.Square)
                    for j in range(4):
                        jj = h * 4 + j
                        nc.tensor.matmul(
                            n2ps[:, jj * 512:jj * 512 + 500],
                            ones_sq[:kw, :],
                            wraw[:kw, j * 500:(j + 1) * 500],
                            start=(k == 0),
                            stop=(k == KTILES - 1),
                        )

            # poly in s = norm^2 (chunks of 500, aligned to the PSUM blocks):
            #   rneg = -(sinh|w|/|w|) = -(1 + s/6 + s^2/120 + s^3/5040)
            #   cm1  = cosh|w| - 1    = s*(1/2 + s/24 + s^2/720)
            for j in range(8):
                s_ps = n2ps[:, j * 512:j * 512 + 500]
                sl = slice(j * 500, (j + 1) * 500)
                a = prep.tile([128, 500], f32, tag="pa", name="pa")
                bt = prep.tile([128, 500], f32, tag="pb", name="pb")
                # rneg chunk
                nc.vector.tensor_scalar(a[:], s_ps, 1.0 / 5040.0, 1.0 / 120.0,
                                        ALU.mult, ALU.add)
                nc.vector.tensor_mul(bt[:], s_ps, a[:])
                nc.vector.tensor_scalar(bt[:], bt[:], 1.0 / 6.0, None, ALU.add)
                nc.vector.tensor_mul(a[:], s_ps, bt[:])
                nc.vector.tensor_scalar(rnegb[:, sl], a[:], 1.0, -1.0,
                                        ALU.add, ALU.mult)

            # pass 2: WT[k] = bf16(rneg * wT[k])  (rows 0-2 of tile 0: zeros*x,
            # then overwritten above -- Tile orders by deps)
            for k in range(KTILES):
                kw = _ktile_rows(k)
                for h in range(2):
                    wraw = prep.tile([128, HW], f32, tag="raw", name="wraw2")
                    nc.sync.dma_start(
                        out=wraw[:kw, :],
                        in_=wT[k * 128:k * 128 + kw, h * HW:(h + 1) * HW])
                    nc.vector.tensor_mul(
                        WTb[k][:kw, h * HW:(h + 1) * HW],
                        wraw[:kw, :], rnegb[:kw, h * HW:(h + 1) * HW])

            # special rows of WT tile 0: [cm1_hi, cm1_lo, cm1_hi]
            # (emitted after the pass-2 mult that zero-fills these rows)
            for j in range(8):
                s_ps = n2ps[:, j * 512:j * 512 + 500]
                sl = slice(j * 500, (j + 1) * 500)
                a = prep.tile([128, 500], f32, tag="pa", name="pa2")
                bt = prep.tile([128, 500], f32, tag="pb", name="pb2")
                lob = prep.tile([1, 500], bf16, tag="lob", name="lob")
                nc.vector.tensor_scalar(a[:], s_ps, 1.0 / 720.0, 1.0 / 24.0,
                                        ALU.mult, ALU.add)
                nc.vector.tensor_mul(bt[:], s_ps, a[:])
                nc.vector.tensor_scalar(bt[:], bt[:], 0.5, None, ALU.add)
                nc.vector.tensor_mul(a[:], s_ps, bt[:])          # a = cm1 (f32)
                nc.vector.tensor_copy(out=WTb[0][0:1, sl], in_=a[0:1, :])   # hi
                nc.vector.tensor_copy(out=bt[0:1, :], in_=WTb[0][0:1, sl])  # hi f32
                nc.vector.scalar_tensor_tensor(
                    out=lob[:], in0=a[0:1, :], scalar=1.0,
                    in1=bt[0:1, :], op0=ALU.mult, op1=ALU.subtract)         # lo
                nc.sync.dma_start(out=WTb[0][1:2, sl], in_=lob[:])
                nc.sync.dma_start(out=WTb[0][2:3, sl], in_=WTb[0][0:1, sl])

            # h: cast f32 -> bf16; tile0 row2 becomes h0_lo = h0 - f32(bf16(h0))
            for k in range(KTILES):
                kw = _ktile_rows(k)
                for h in range(2):
                    hraw = prep.tile([128, HS], f32, tag="raw", name="hraw")
                    nc.sync.dma_start(
                        out=hraw[:kw, :],
                        in_=hT[k * 128:k * 128 + kw, h * HS:(h + 1) * HS])
                    nc.vector.tensor_copy(out=hTbf[k][:kw, h * HS:(h + 1) * HS],
                                          in_=hraw[:kw, :])
                    if k == 0:
                        for q in range(2):
                            cs = slice(h * HS + q * 1024, h * HS + (q + 1) * 1024)
                            qs = slice(q * 1024, (q + 1) * 1024)
                            hhi = prep.tile([1, 1024], f32, tag="hhi", name="hhi")
                            hlo = prep.tile([1, 1024], bf16, tag="hlo", name="hlo")
                            nc.vector.tensor_copy(out=hhi[:], in_=hTbf[0][0:1, cs])
                            nc.vector.scalar_tensor_tensor(
                                out=hlo[:], in0=hraw[0:1, qs],
                                scalar=1.0, in1=hhi[:],
                                op0=ALU.mult, op1=ALU.subtract)
                            nc.sync.dma_start(out=hTbf[0][2:3, cs], in_=hlo[:])

            # bias 2*h0 in column layout
            h0ct = prep1.tile([128, MT], f32, tag="h0ct")
            nc.sync.dma_start(out=h0ct[:], in_=h0c[:, :])
            nc.vector.tensor_scalar_mul(b2[:], h0ct[:], 2.0)

        # ---------------- main loop ----------------
        with tc.tile_pool(name="mpsum", bufs=2, space="PSUM") as mpsum, \
             tc.tile_pool(name="pd0", bufs=2) as pd0, \
             tc.tile_pool(name="pu", bufs=1) as pu, \
             tc.tile_pool(name="pot", bufs=2) as pot:

            for m in range(MT):
                ms = m * 128
                for half in range(2):
                    c0 = half * HALF
                    F = min(HALF, VLOC - c0)          # 2048 or 1952
                    ps = mpsum.tile([128, HALF], f32, tag="ps", name="ps")
                    for k in range(KTILES):
                        kw = _ktile_rows(k)
                        lhsT = hTbf[k][:kw, ms:ms + 128]
                        for j in range(4):
                            n0 = c0 + j * NCHUNK
                            nw = min(NCHUNK, VLOC - n0)
                            if nw <= 0:
                                continue
                            nc.tensor.matmul(
                                ps[:, j * NCHUNK:j * NCHUNK + nw],
                                lhsT,
                                WTb[k][:kw, n0:n0 + nw],
                                start=(k == 0),
                                stop=(k == KTILES - 1),
                            )
                    bias = b2[:, m:m + 1]
                    d0 = pd0.tile([128, HALF], f32, tag="d0", name="d0")
                    u = pu.tile([128, HALF], f32, tag="u", name="u")
                    ot = pot.tile([128, HALF], f32, tag="ot", name="ot")
                    # d0 = ln(2y + 2h0) = ln(2x)
                    nc.scalar.activation(d0[:, :F], ps[:, :F], AF.Ln,
                                         bias=bias, scale=2.0)
                    # u = exp(-2*d0) = 1/(4x^2)
                    nc.scalar.activation(u[:, :F], d0[:, :F], AF.Exp, scale=-2.0)
                    # u := d0 - u   (= acosh(x) up to O(x^-4))
                    nc.vector.scalar_tensor_tensor(
                        out=u[:, :F], in0=u[:, :F], scalar=-1.0, in1=d0[:, :F],
                        op0=ALU.mult, op1=ALU.add,
                    )
                    # d0 := u^2 = d^2
                    nc.scalar.activation(d0[:, :F], u[:, :F], AF.Square)
                    # ot = -tau * d^2
                    nc.gpsimd.tensor_scalar_mul(ot[:, :F], d0[:, :F], neg_tau)
                    nc.sync.dma_start(
                        out=out[ms:ms + 128, c0:c0 + F], in_=ot[:, :F]
                    )

    nc.compile()
    return nc


def _stage_inputs(hidden_states, weight, logit_scale):
    h = np.ascontiguousarray(hidden_states.reshape(S, N + 1), dtype=np.float32)
    h0 = h[:, 0]
    hT_host = np.empty((KDIM, S), dtype=np.float32)
    hT_host[0] = h0
    hT_host[1] = h0
    hT_host[2] = h0
    hT_host[3:] = h[:, 1:].T
    h0c_host = np.ascontiguousarray(h0.reshape(MT, 128).T)

    w = np.asarray(weight, dtype=np.float32)
    in_maps = []
    for c in range(NCORES):
        wT_host = np.zeros((KDIM, VLOC), dtype=np.float32)
        wT_host[3:] = w[c * VLOC:(c + 1) * VLOC, :].T
        in_maps.append({
            "hT": hT_host,
            "h0c": h0c_host,
            "wT": wT_host,
        })
    tau = float(np.clip(np.float32(logit_scale), 0.01, 2.5))
    return in_maps, tau


def kernel(hidden_states, weight, logit_scale):
    global LAST_EXEC_NS, LAST_RESULTS
    from concourse import bass_utils

    in_maps, tau = _stage_inputs(hidden_states, weight, logit_scale)
    key = round(tau, 9)
    if key not in _BUILD_CACHE:
        _BUILD_CACHE[key] = _build(tau)
    nc = _BUILD_CACHE[key]

    res = bass_utils.run_bass_kernel_spmd(nc, in_maps, core_ids=list(range(NCORES)))
    LAST_EXEC_NS = res.exec_time_ns if res.exec_time_ns else res.mean_exec_time_ns
    LAST_RESULTS = res
    outs = [res.results[c]["out"] for c in range(NCORES)]
    logits = np.concatenate(outs, axis=1).reshape(B, L, V)
    return np.ascontiguousarray(logits.astype(np.float32))

_RUNNER_CACHE = {}


def _make_runner(nc, donate):
    """Cached jitted 8-core runner mirroring bass2jax.run_bass_via_pjrt."""
    import jax
    import concourse.mybir as mybir
    from concourse import bass2jax
    from jax.experimental.shard_map import shard_map
    from jax.sharding import Mesh, PartitionSpec

    bass2jax.install_neuronx_cc_hook()

    pname = nc.partition_id_tensor.name if nc.partition_id_tensor else None
    in_names, out_names, out_avals, zero_outs = [], [], [], []
    for alloc in nc.m.functions[0].allocations:
        if not isinstance(alloc, mybir.MemoryLocationSet):
            continue
        name = alloc.memorylocations[0].name
        if alloc.kind == "ExternalInput":
            if name != pname:
                in_names.append(name)
        elif alloc.kind == "ExternalOutput":
            out_names.append(name)
            shape = tuple(alloc.tensor_shape)
            dtype = mybir.dt.np(alloc.dtype)
            out_avals.append(jax.core.ShapedArray(shape, dtype))
            zero_outs.append(np.zeros(shape, dtype))
    n_params = len(in_names)
    all_in_names = in_names + out_names
    if pname is not None:
        all_in_names = all_in_names + [pname]

    def _body(*args):
        operands = list(args)
        if pname is not None:
            operands.append(bass2jax.partition_id_tensor())
        outs = bass2jax._bass_exec_p.bind(
            *operands,
            out_avals=tuple(out_avals),
            in_names=tuple(all_in_names),
            out_names=tuple(out_names),
            lowering_input_output_aliases=(),
            sim_require_finite=True,
            sim_require_nnan=True,
            nc=nc,
        )
        return tuple(outs)

    devices = jax.devices()[:NCORES]
    mesh = Mesh(np.asarray(devices), ("core",))
    n_outs = len(out_names)
    in_specs = (PartitionSpec("core"),) * (n_params + n_outs)
    out_specs = (PartitionSpec("core"),) * n_outs
    jit_kwargs = dict(keep_unused=True)
    if donate:
        jit_kwargs["donate_argnums"] = tuple(range(n_params, n_params + n_outs))
    fn = jax.jit(
        shard_map(_body, mesh=mesh, in_specs=in_specs, out_specs=out_specs,
                  check_rep=False),
        **jit_kwargs,
    )
    return fn, in_names, out_names, zero_outs


def run_and_bench(hidden_states, weight, logit_scale, n_timed=8):
    """Run once for output + time warm device-resident executions.

    Returns (logits, per_call_wall_ns_list).
    """
    import jax
    in_maps, tau = _stage_inputs(hidden_states, weight, logit_scale)
    key = round(tau, 9)
    if key not in _BUILD_CACHE:
        _BUILD_CACHE[key] = _build(tau)
    nc = _BUILD_CACHE[key]
    if key not in _RUNNER_CACHE:
        _RUNNER_CACHE[key] = _make_runner(nc, donate=False)
    fn, in_names, out_names, zero_outs = _RUNNER_CACHE[key]

    concat_in = [
        np.concatenate([np.asarray(in_maps[c][nm]) for c in range(NCORES)], axis=0)
        for nm in in_names
    ]
    dev_in = [jax.device_put(a) for a in concat_in]
    dev_zeros = [
        jax.device_put(np.concatenate([z] * NCORES, axis=0)) for z in zero_outs
    ]
    outs = fn(*dev_in, *dev_zeros)
    jax.block_until_ready(outs)

    import time as _t
    walls = []
    for _ in range(n_timed):
        t0 = _t.perf_counter()
        o = fn(*dev_in, *dev_zeros)
        jax.block_until_ready(o)
        walls.append((_t.perf_counter() - t0) * 1e9)

    full = np.asarray(outs[out_names.index("out")])      # [8*S, VLOC]
    parts = [full[c * S:(c + 1) * S] for c in range(NCORES)]
    logits = np.concatenate(parts, axis=1).reshape(B, L, V)
    return np.ascontiguousarray(logits.astype(np.float32)), walls

